# revision 8
# baseline (speedup 1.0000x reference)
"""GATv2 2-layer encoder on 8 TRN2 NeuronCores.

Strategy: destination-node sharding. Nodes are bin-packed into 392 tiles of
128 slots each (balancing in-edge counts), 49 tiles per core. All edges
(incl. self-loops) are grouped by the tile owning their destination; each
tile's edges are padded to BPT blocks of 128. Per edge-block the kernel
gathers xl[src], e[rel], xr[dst] rows (indirect DMA, summed in the DMA
datapath), applies leaky-relu + per-head att dot to get logits, exp (softmax
without max-subtraction — logits are O(1)), and scatter-adds the weighted
source features into the tile's PSUM accumulator with a one-hot matmul.
Segment softmax needs no cross-core traffic; the only collectives are
AllGathers of the per-core node-feature table shards between layers.
"""
import sys
import heapq

import numpy as np

sys.path.insert(0, "/opt/trn_rl_repo")

import ml_dtypes  # noqa: E402
import concourse.bass as bass  # noqa: E402
import concourse.tile as tile  # noqa: E402
from concourse import bacc, mybir  # noqa: E402
from concourse.bass_utils import run_bass_kernel_spmd  # noqa: E402
from concourse.masks import make_identity  # noqa: E402

N, E, R = 50000, 400000, 500
IN, HID, H, OUT = 128, 64, 4, 128
HC1, HC2 = H * HID, H * OUT  # 256, 512
W = 8            # cores
P = 128          # partitions / tile slots / edge-block size
NT = 49          # node tiles per core
TILES = W * NT   # 392
NSLOT = TILES * P  # 50176
SHARD = NT * P   # 6272 rows per core
RPAD = 512       # padded relation table rows (row R = zeros for self-loops)

F32 = mybir.dt.float32
BF16 = mybir.dt.bfloat16
I32 = mybir.dt.int32
BF = ml_dtypes.bfloat16


def _preprocess(edge_index):
    """Self-loops, balanced node->tile binning, per-core block index arrays."""
    src = np.asarray(edge_index[0], dtype=np.int64)
    rel = np.asarray(edge_index[1], dtype=np.int64)
    dst = np.asarray(edge_index[2], dtype=np.int64)
    loop = np.arange(N, dtype=np.int64)
    src_f = np.concatenate([src, loop])
    dst_f = np.concatenate([dst, loop])
    rel_f = np.concatenate([rel, np.full(N, R, dtype=np.int64)])

    deg = np.bincount(dst_f, minlength=N)

    # Greedy balanced binning: highest-degree node to lightest non-full tile.
    order = np.argsort(-deg, kind="stable")
    tile_of = np.empty(N, np.int64)
    slot_of = np.empty(N, np.int64)
    heap = [(0, t) for t in range(TILES)]
    heapq.heapify(heap)
    counts = np.zeros(TILES, np.int64)
    loads = np.zeros(TILES, np.int64)
    for n in order:
        while True:
            load, t = heapq.heappop(heap)
            if counts[t] < P:
                break
        tile_of[n] = t
        slot_of[n] = counts[t]
        counts[t] += 1
        loads[t] += deg[n]
        if counts[t] < P:
            heapq.heappush(heap, (loads[t], t))

    perm_pos = tile_of * P + slot_of  # node -> row in permuted table layout

    bpt = max(1, int(-(-loads.max() // P)))  # blocks per tile (uniform)
    nblk = NT * bpt
    cap = bpt * P

    # Edge slots per tile, padded to cap.
    et = tile_of[dst_f]
    eorder = np.argsort(et, kind="stable")
    et_s = et[eorder]
    starts = np.searchsorted(et_s, np.arange(TILES))
    ends = np.searchsorted(et_s, np.arange(TILES), side="right")

    src_a = np.zeros((TILES, cap), np.int64)
    rel_a = np.full((TILES, cap), R, np.int64)
    dst_a = np.zeros((TILES, cap), np.int64)
    seg_a = np.full((TILES, cap), 999, np.int64)  # 999 => zero Q row (pad)
    for t in range(TILES):
        idx = eorder[starts[t]:ends[t]]
        k = idx.shape[0]
        src_a[t, :k] = src_f[idx]
        rel_a[t, :k] = rel_f[idx]
        dst_a[t, :k] = dst_f[idx]
        seg_a[t, :k] = slot_of[dst_f[idx]]

    # Per-core arrays.
    gidx1 = np.zeros((W, P, nblk * 3), np.int32)
    gidx2 = np.zeros((W, P, nblk * 3), np.int32)
    qh = np.zeros((W, nblk * P, P), BF)
    node_of_slot = np.full(NSLOT, N, np.int64)  # pad slots -> dummy
    for n in range(N):
        node_of_slot[perm_pos[n]] = n
    eye = np.eye(P, dtype=BF)
    zrow = np.zeros(P, BF)
    for c in range(W):
        for t in range(NT):
            g = c * NT + t
            s3 = src_a[g].reshape(bpt, P)
            r3 = rel_a[g].reshape(bpt, P)
            d3 = dst_a[g].reshape(bpt, P)
            sg3 = seg_a[g].reshape(bpt, P)
            for j in range(bpt):
                b = t * bpt + j
                gidx1[c, :, 3 * b + 0] = s3[j]
                gidx1[c, :, 3 * b + 1] = r3[j]
                gidx1[c, :, 3 * b + 2] = d3[j]
                rows = qh[c, b * P:(b + 1) * P]
                valid = sg3[j] < P
                rows[valid] = eye[sg3[j][valid]]
                rows[~valid] = zrow
        # vectorized gidx2 fill
        g0 = c * NT
        s_all = src_a[g0:g0 + NT].reshape(NT * bpt, P)
        r_all = rel_a[g0:g0 + NT].reshape(NT * bpt, P)
        d_all = dst_a[g0:g0 + NT].reshape(NT * bpt, P)
        gidx2[c, :, 0::3] = perm_pos[s_all].T
        gidx2[c, :, 1::3] = r_all.T
        gidx2[c, :, 2::3] = perm_pos[d_all].T

    return dict(
        bpt=bpt, nblk=nblk, perm_pos=perm_pos, node_of_slot=node_of_slot,
        gidx1=gidx1, gidx2=gidx2, qh=qh,
    )


def _build(bpt):
    nblk = NT * bpt
    nc = bacc.Bacc("TRN2", target_bir_lowering=False, debug=False, num_devices=W)

    # ---- per-core inputs
    x_shard = nc.declare_dram_parameter("x_shard", [SHARD, IN], F32, isOutput=False)
    gidx1 = nc.declare_dram_parameter("gidx1", [P, nblk * 3], I32, isOutput=False)
    gidx2 = nc.declare_dram_parameter("gidx2", [P, nblk * 3], I32, isOutput=False)
    qh = nc.declare_dram_parameter("qh", [nblk * P, P], BF16, isOutput=False)
    # ---- replicated inputs
    rel_pad = nc.declare_dram_parameter("rel_pad", [RPAD, IN], F32, isOutput=False)
    wl1 = nc.declare_dram_parameter("wl1", [IN, HC1], F32, isOutput=False)
    wr1 = nc.declare_dram_parameter("wr1", [IN, HC1], F32, isOutput=False)
    we1 = nc.declare_dram_parameter("we1", [IN, HC1], F32, isOutput=False)
    att1f = nc.declare_dram_parameter("att1f", [1, HC1], F32, isOutput=False)
    eb1 = nc.declare_dram_parameter("eb1", [1, HC1], F32, isOutput=False)
    ob1 = nc.declare_dram_parameter("ob1", [1, HC1], F32, isOutput=False)
    wl2 = nc.declare_dram_parameter("wl2", [HC1, HC2], F32, isOutput=False)
    wr2 = nc.declare_dram_parameter("wr2", [HC1, HC2], F32, isOutput=False)
    we2 = nc.declare_dram_parameter("we2", [IN, HC2], F32, isOutput=False)
    att2f = nc.declare_dram_parameter("att2f", [1, HC2], F32, isOutput=False)
    eb2 = nc.declare_dram_parameter("eb2", [1, HC2], F32, isOutput=False)
    ob2 = nc.declare_dram_parameter("ob2", [1, OUT], F32, isOutput=False)
    out_p = nc.declare_dram_parameter("out", [SHARD, OUT], F32, isOutput=True)

    # ---- internal DRAM
    e1t = nc.dram_tensor("e1t", [RPAD, HC1], BF16)
    e2t = nc.dram_tensor("e2t", [RPAD, HC2], BF16)
    xl_shard = nc.dram_tensor("xl_shard", [SHARD, HC1], BF16)
    xr_shard = nc.dram_tensor("xr_shard", [SHARD, HC1], BF16)
    xl1_full = nc.dram_tensor("xl1_full", [NSLOT, HC1], BF16, addr_space="Shared")
    xr1_full = nc.dram_tensor("xr1_full", [NSLOT, HC1], BF16, addr_space="Shared")
    h_shard = nc.dram_tensor("h_shard", [SHARD, HC1], BF16)
    xl2_shard = nc.dram_tensor("xl2_shard", [SHARD, HC2], BF16)
    xr2_shard = nc.dram_tensor("xr2_shard", [SHARD, HC2], BF16)
    xl2_full = nc.dram_tensor("xl2_full", [NSLOT, HC2], BF16, addr_space="Shared")
    xr2_full = nc.dram_tensor("xr2_full", [NSLOT, HC2], BF16, addr_space="Shared")

    RG = [list(range(W))]
    IOA = bass.IndirectOffsetOnAxis

    with tile.TileContext(nc) as tc:
        with (
            tc.tile_pool(name="const", bufs=1) as cp,
            tc.tile_pool(name="work", bufs=3) as wp,
            tc.tile_pool(name="pstp", bufs=2, space="PSUM") as pp_tp,
            tc.tile_pool(name="psb", bufs=1, space="PSUM") as pp_b,
            tc.tile_pool(name="psacc", bufs=2, space="PSUM") as pa,
        ):
            # ================= consts =================
            ident = cp.tile([P, P], BF16)
            make_identity(nc, ident[:])
            wl1b = cp.tile([IN, HC1], BF16, tag="wl1b")
            nc.gpsimd.dma_start(out=wl1b[:], in_=wl1[:])
            wr1b = cp.tile([IN, HC1], BF16, tag="wr1b")
            nc.gpsimd.dma_start(out=wr1b[:], in_=wr1[:])
            we1b = cp.tile([IN, HC1], BF16, tag="we1b")
            nc.gpsimd.dma_start(out=we1b[:], in_=we1[:])
            we2b = cp.tile([IN, HC2], BF16, tag="we2b")
            nc.gpsimd.dma_start(out=we2b[:], in_=we2[:])
            wl2b = []
            wr2b = []
            for k in range(2):
                wl2bk = cp.tile([P, HC2], BF16, tag=f"wl2b{k}")
                nc.gpsimd.dma_start(out=wl2bk[:], in_=wl2[k * P:(k + 1) * P, :])
                wl2b.append(wl2bk)
                wr2bk = cp.tile([P, HC2], BF16, tag=f"wr2b{k}")
                nc.gpsimd.dma_start(out=wr2bk[:], in_=wr2[k * P:(k + 1) * P, :])
                wr2b.append(wr2bk)
            attB1 = cp.tile([P, HC1], BF16, tag="attB1")
            nc.gpsimd.dma_start(out=attB1[:], in_=att1f[:].to_broadcast([P, HC1]))
            attB2 = cp.tile([P, HC2], BF16, tag="attB2")
            nc.gpsimd.dma_start(out=attB2[:], in_=att2f[:].to_broadcast([P, HC2]))
            eb1B = cp.tile([P, HC1], F32, tag="eb1B")
            nc.sync.dma_start(out=eb1B[:], in_=eb1[:].to_broadcast([P, HC1]))
            ob1B = cp.tile([P, HC1], BF16, tag="ob1B")
            nc.gpsimd.dma_start(out=ob1B[:], in_=ob1[:].to_broadcast([P, HC1]))
            eb2B = cp.tile([P, HC2], F32, tag="eb2B")
            nc.sync.dma_start(out=eb2B[:], in_=eb2[:].to_broadcast([P, HC2]))
            ob2B = cp.tile([P, OUT], F32, tag="ob2B")
            nc.sync.dma_start(out=ob2B[:], in_=ob2[:].to_broadcast([P, OUT]))
            gidx1_t = cp.tile([P, nblk * 3], I32, tag="gidx1_t")
            nc.sync.dma_start(out=gidx1_t[:], in_=gidx1[:])
            gidx2_t = cp.tile([P, nblk * 3], I32, tag="gidx2_t")
            nc.sync.dma_start(out=gidx2_t[:], in_=gidx2[:])

            # ================= e-tables =================
            for k in range(RPAD // P):
                rk = wp.tile([P, IN], BF16, tag="rk")
                nc.gpsimd.dma_start(out=rk[:], in_=rel_pad[k * P:(k + 1) * P, :])
                tp = pp_tp.tile([P, P], BF16, tag="tp")
                nc.tensor.transpose(tp[:], rk[:], ident[:])
                rT = wp.tile([P, IN], BF16, tag="rT")
                nc.vector.tensor_copy(rT[:], tp[:])
                psE1 = pp_b.tile([P, HC2], F32, tag="psb")
                nc.tensor.matmul(psE1[:, 0:HC1], lhsT=rT[:], rhs=we1b[:],
                                 start=True, stop=True)
                e1sb = wp.tile([P, HC1], BF16, tag="e1sb")
                nc.vector.tensor_tensor(out=e1sb[:], in0=psE1[:, 0:HC1], in1=eb1B[:],
                                        op=mybir.AluOpType.add)
                nc.sync.dma_start(out=e1t[k * P:(k + 1) * P, :], in_=e1sb[:])
                psE2 = pp_b.tile([P, HC2], F32, tag="psb")
                nc.tensor.matmul(psE2[:], lhsT=rT[:], rhs=we2b[:], start=True, stop=True)
                e2sb = wp.tile([P, HC2], BF16, tag="e2sb")
                nc.vector.tensor_tensor(out=e2sb[:], in0=psE2[:], in1=eb2B[:],
                                        op=mybir.AluOpType.add)
                nc.sync.dma_start(out=e2t[k * P:(k + 1) * P, :], in_=e2sb[:])

            # ================= xl1/xr1 shard build =================
            for t in range(NT):
                xt = wp.tile([P, IN], BF16, tag="xt")
                nc.gpsimd.dma_start(out=xt[:], in_=x_shard[t * P:(t + 1) * P, :])
                tp2 = pp_tp.tile([P, P], BF16, tag="tp")
                nc.tensor.transpose(tp2[:], xt[:], ident[:])
                xT = wp.tile([P, IN], BF16, tag="xT")
                nc.vector.tensor_copy(xT[:], tp2[:])
                psC = pp_b.tile([P, HC2], F32, tag="psb")
                nc.tensor.matmul(psC[:, 0:HC1], lhsT=xT[:], rhs=wl1b[:],
                                 start=True, stop=True)
                nc.tensor.matmul(psC[:, HC1:HC2], lhsT=xT[:], rhs=wr1b[:],
                                 start=True, stop=True)
                xlsb = wp.tile([P, HC1], BF16, tag="xlsb")
                nc.scalar.activation(xlsb[:], psC[:, 0:HC1],
                                     mybir.ActivationFunctionType.Copy)
                nc.sync.dma_start(out=xl_shard[t * P:(t + 1) * P, :], in_=xlsb[:])
                xrsb = wp.tile([P, HC1], BF16, tag="xrsb")
                nc.scalar.activation(xrsb[:], psC[:, HC1:HC2],
                                     mybir.ActivationFunctionType.Copy)
                nc.sync.dma_start(out=xr_shard[t * P:(t + 1) * P, :], in_=xrsb[:])

            nc.gpsimd.collective_compute(
                "AllGather", mybir.AluOpType.bypass,
                ins=[xl_shard[:]], outs=[xl1_full[:]], replica_groups=RG)
            nc.gpsimd.collective_compute(
                "AllGather", mybir.AluOpType.bypass,
                ins=[xr_shard[:]], outs=[xr1_full[:]], replica_groups=RG)

            # ================= layer-1 edges =================
            for t in range(NT):
                acc1 = pa.tile([P, HC1 + 4], F32, tag="accF")
                for j in range(bpt):
                    b = t * bpt + j
                    Qb = wp.tile([P, P], BF16, tag="Qb")
                    nc.sync.dma_start(out=Qb[:], in_=qh[b * P:(b + 1) * P, :])
                    Gl = wp.tile([P, HC1], BF16, tag="Gl")
                    nc.gpsimd.indirect_dma_start(
                        out=Gl[:], out_offset=None, in_=xl1_full[:],
                        in_offset=IOA(ap=gidx1_t[:, 3 * b:3 * b + 1], axis=0))
                    M = wp.tile([P, HC1], BF16, tag="M")
                    nc.gpsimd.indirect_dma_start(
                        out=M[:], out_offset=None, in_=e1t[:],
                        in_offset=IOA(ap=gidx1_t[:, 3 * b + 1:3 * b + 2], axis=0))
                    nc.gpsimd.indirect_dma_start(
                        out=M[:], out_offset=None, in_=xr1_full[:],
                        in_offset=IOA(ap=gidx1_t[:, 3 * b + 2:3 * b + 3], axis=0),
                        compute_op=mybir.AluOpType.add)
                    nc.gpsimd.indirect_dma_start(
                        out=M[:], out_offset=None, in_=xl1_full[:],
                        in_offset=IOA(ap=gidx1_t[:, 3 * b:3 * b + 1], axis=0),
                        compute_op=mybir.AluOpType.add)
                    Mr = wp.tile([P, HC1], BF16, tag="Mr")
                    nc.scalar.activation(Mr[:], M[:],
                                         mybir.ActivationFunctionType.Prelu, alpha=0.2)
                    T = wp.tile([P, HC1], BF16, tag="T")
                    nc.vector.tensor_tensor(out=T[:], in0=Mr[:], in1=attB1[:],
                                            op=mybir.AluOpType.mult)
                    logit = wp.tile([P, H], F32, tag="logit")
                    nc.vector.tensor_reduce(
                        out=logit[:], in_=T[:].rearrange("p (h c) -> p h c", h=H),
                        axis=mybir.AxisListType.X, op=mybir.AluOpType.add)
                    wf = wp.tile([P, H], F32, tag="wf")
                    nc.scalar.activation(wf[:], logit[:],
                                         mybir.ActivationFunctionType.Exp)
                    Rt = wp.tile([P, HC1 + 4], BF16, tag="Rt")
                    nc.scalar.activation(Rt[:, HC1:HC1 + 4], logit[:],
                                         mybir.ActivationFunctionType.Exp)
                    for hh in range(H):
                        nc.scalar.activation(
                            Rt[:, hh * HID:(hh + 1) * HID], Gl[:, hh * HID:(hh + 1) * HID],
                            mybir.ActivationFunctionType.Copy,
                            scale=wf[:, hh:hh + 1])
                    nc.tensor.matmul(acc1[:], lhsT=Qb[:], rhs=Rt[:],
                                     start=(j == 0), stop=(j == bpt - 1))
                # epilogue: h = acc/denom + bias
                rec = wp.tile([P, H], F32, tag="rec")
                nc.vector.reciprocal(rec[:], acc1[:, HC1:HC1 + 4])
                htmp = wp.tile([P, HC1], BF16, tag="htmp")
                for hh in range(H):
                    nc.scalar.activation(
                        htmp[:, hh * HID:(hh + 1) * HID],
                        acc1[:, hh * HID:(hh + 1) * HID],
                        mybir.ActivationFunctionType.Copy, scale=rec[:, hh:hh + 1])
                hsb = wp.tile([P, HC1], BF16, tag="hsb")
                nc.vector.tensor_tensor(out=hsb[:], in0=htmp[:], in1=ob1B[:],
                                        op=mybir.AluOpType.add)
                nc.sync.dma_start(out=h_shard[t * P:(t + 1) * P, :], in_=hsb[:])

            # ================= xl2/xr2 build =================
            for t in range(NT):
                ht = wp.tile([P, HC1], BF16, tag="ht")
                nc.sync.dma_start(out=ht[:], in_=h_shard[t * P:(t + 1) * P, :])
                hT = []
                for k in range(2):
                    tp3 = pp_tp.tile([P, P], BF16, tag="tp")
                    nc.tensor.transpose(tp3[:], ht[:, k * P:(k + 1) * P], ident[:])
                    hTk = wp.tile([P, P], BF16, tag=f"hT{k}")
                    nc.vector.tensor_copy(hTk[:], tp3[:])
                    hT.append(hTk)
                ps2l = pp_b.tile([P, HC2], F32, tag="psb")
                for k in range(2):
                    nc.tensor.matmul(ps2l[:], lhsT=hT[k][:], rhs=wl2b[k][:],
                                     start=(k == 0), stop=(k == 1))
                xl2sb = wp.tile([P, HC2], BF16, tag="xl2sb")
                nc.scalar.activation(xl2sb[:], ps2l[:], mybir.ActivationFunctionType.Copy)
                nc.sync.dma_start(out=xl2_shard[t * P:(t + 1) * P, :], in_=xl2sb[:])
                ps2r = pp_b.tile([P, HC2], F32, tag="psb")
                for k in range(2):
                    nc.tensor.matmul(ps2r[:], lhsT=hT[k][:], rhs=wr2b[k][:],
                                     start=(k == 0), stop=(k == 1))
                xr2sb = wp.tile([P, HC2], BF16, tag="xr2sb")
                nc.scalar.activation(xr2sb[:], ps2r[:], mybir.ActivationFunctionType.Copy)
                nc.sync.dma_start(out=xr2_shard[t * P:(t + 1) * P, :], in_=xr2sb[:])

            nc.gpsimd.collective_compute(
                "AllGather", mybir.AluOpType.bypass,
                ins=[xl2_shard[:]], outs=[xl2_full[:]], replica_groups=RG)
            nc.gpsimd.collective_compute(
                "AllGather", mybir.AluOpType.bypass,
                ins=[xr2_shard[:]], outs=[xr2_full[:]], replica_groups=RG)

            # ================= layer-2 edges =================
            for t in range(NT):
                acc2 = pa.tile([P, HC2], F32, tag="accF")
                accd = pa.tile([P, 4], F32, tag="accD")
                for j in range(bpt):
                    b = t * bpt + j
                    Qb = wp.tile([P, P], BF16, tag="Qb")
                    nc.sync.dma_start(out=Qb[:], in_=qh[b * P:(b + 1) * P, :])
                    Gl2 = wp.tile([P, HC2], BF16, tag="Gl2")
                    nc.gpsimd.indirect_dma_start(
                        out=Gl2[:], out_offset=None, in_=xl2_full[:],
                        in_offset=IOA(ap=gidx2_t[:, 3 * b:3 * b + 1], axis=0))
                    M2 = wp.tile([P, HC2], BF16, tag="M2")
                    nc.gpsimd.indirect_dma_start(
                        out=M2[:], out_offset=None, in_=e2t[:],
                        in_offset=IOA(ap=gidx2_t[:, 3 * b + 1:3 * b + 2], axis=0))
                    nc.gpsimd.indirect_dma_start(
                        out=M2[:], out_offset=None, in_=xr2_full[:],
                        in_offset=IOA(ap=gidx2_t[:, 3 * b + 2:3 * b + 3], axis=0),
                        compute_op=mybir.AluOpType.add)
                    nc.gpsimd.indirect_dma_start(
                        out=M2[:], out_offset=None, in_=xl2_full[:],
                        in_offset=IOA(ap=gidx2_t[:, 3 * b:3 * b + 1], axis=0),
                        compute_op=mybir.AluOpType.add)
                    Mr2 = wp.tile([P, HC2], BF16, tag="Mr2")
                    nc.scalar.activation(Mr2[:], M2[:],
                                         mybir.ActivationFunctionType.Prelu, alpha=0.2)
                    T2 = wp.tile([P, HC2], BF16, tag="T2")
                    nc.vector.tensor_tensor(out=T2[:], in0=Mr2[:], in1=attB2[:],
                                            op=mybir.AluOpType.mult)
                    logit2 = wp.tile([P, H], F32, tag="logit2")
                    nc.vector.tensor_reduce(
                        out=logit2[:], in_=T2[:].rearrange("p (h c) -> p h c", h=H),
                        axis=mybir.AxisListType.X, op=mybir.AluOpType.add)
                    wf2 = wp.tile([P, H], F32, tag="wf2")
                    nc.scalar.activation(wf2[:], logit2[:],
                                         mybir.ActivationFunctionType.Exp)
                    R2 = wp.tile([P, HC2 + 4], BF16, tag="R2")
                    nc.scalar.activation(R2[:, HC2:HC2 + 4], logit2[:],
                                         mybir.ActivationFunctionType.Exp)
                    for hh in range(H):
                        nc.scalar.activation(
                            R2[:, hh * OUT:(hh + 1) * OUT], Gl2[:, hh * OUT:(hh + 1) * OUT],
                            mybir.ActivationFunctionType.Copy,
                            scale=wf2[:, hh:hh + 1])
                    nc.tensor.matmul(acc2[:], lhsT=Qb[:], rhs=R2[:, 0:HC2],
                                     start=(j == 0), stop=(j == bpt - 1))
                    nc.tensor.matmul(accd[:], lhsT=Qb[:], rhs=R2[:, HC2:HC2 + 4],
                                     start=(j == 0), stop=(j == bpt - 1))
                # epilogue: out = mean_h(acc_h/denom_h) + bias
                rec2 = wp.tile([P, H], F32, tag="rec2")
                nc.vector.reciprocal(rec2[:], accd[:])
                rec4 = wp.tile([P, H], F32, tag="rec4")
                nc.vector.tensor_scalar_mul(rec4[:], rec2[:], 0.25)
                hsum = []
                for hh in range(H):
                    ho = wp.tile([P, OUT], F32, tag=f"ho{hh}")
                    nc.scalar.activation(
                        ho[:], acc2[:, hh * OUT:(hh + 1) * OUT],
                        mybir.ActivationFunctionType.Copy, scale=rec4[:, hh:hh + 1])
                    hsum.append(ho)
                s01 = wp.tile([P, OUT], F32, tag="s01")
                nc.vector.tensor_tensor(out=s01[:], in0=hsum[0][:], in1=hsum[1][:],
                                        op=mybir.AluOpType.add)
                s23 = wp.tile([P, OUT], F32, tag="s23")
                nc.vector.tensor_tensor(out=s23[:], in0=hsum[2][:], in1=hsum[3][:],
                                        op=mybir.AluOpType.add)
                s0123 = wp.tile([P, OUT], F32, tag="s0123")
                nc.vector.tensor_tensor(out=s0123[:], in0=s01[:], in1=s23[:],
                                        op=mybir.AluOpType.add)
                osb = wp.tile([P, OUT], F32, tag="osb")
                nc.vector.tensor_tensor(out=osb[:], in0=s0123[:], in1=ob2B[:],
                                        op=mybir.AluOpType.add)
                nc.sync.dma_start(out=out_p[t * P:(t + 1) * P, :], in_=osb[:])

    nc.compile()
    return nc


_CACHE = {}


def kernel(x, edge_index, relations,
           Wl1, bl1, Wr1, br1, We1, att1, bias1,
           Wl2, bl2, Wr2, br2, We2, att2, bias2, **_unused):
    x = np.asarray(x, np.float32)
    edge_index = np.asarray(edge_index)
    relations = np.asarray(relations, np.float32)

    pre = _preprocess(edge_index)
    bpt = pre["bpt"]

    if bpt not in _CACHE:
        _CACHE[bpt] = _build(bpt)
    nc = _CACHE[bpt]

    x_pad = np.zeros((NSLOT, IN), np.float32)
    x_pad[:N] = x
    rel_pad = np.zeros((RPAD, IN), np.float32)
    rel_pad[:R] = relations

    f32 = np.float32
    rep = dict(
        rel_pad=rel_pad,
        wl1=np.asarray(Wl1, f32), wr1=np.asarray(Wr1, f32), we1=np.asarray(We1, f32),
        att1f=np.asarray(att1, f32).reshape(1, HC1),
        eb1=(np.asarray(bl1, f32) + np.asarray(br1, f32)).reshape(1, HC1),
        ob1=(np.asarray(bl1, f32) + np.asarray(bias1, f32)).reshape(1, HC1),
        wl2=np.asarray(Wl2, f32), wr2=np.asarray(Wr2, f32), we2=np.asarray(We2, f32),
        att2f=np.asarray(att2, f32).reshape(1, HC2),
        eb2=(np.asarray(bl2, f32) + np.asarray(br2, f32)).reshape(1, HC2),
        ob2=(np.asarray(bl2, f32).reshape(H, OUT).mean(axis=0)
             + np.asarray(bias2, f32)).reshape(1, OUT),
    )
    in_maps = []
    for c in range(W):
        m = dict(rep)
        m["x_shard"] = np.ascontiguousarray(x_pad[c * SHARD:(c + 1) * SHARD])
        m["gidx1"] = np.ascontiguousarray(pre["gidx1"][c])
        m["gidx2"] = np.ascontiguousarray(pre["gidx2"][c])
        m["qh"] = np.ascontiguousarray(pre["qh"][c])
        in_maps.append(m)

    res = run_bass_kernel_spmd(nc, in_maps, list(range(W)))
    cat = np.concatenate([res.results[c]["out"] for c in range(W)], axis=0)
    return np.ascontiguousarray(cat[pre["perm_pos"]])


if __name__ == "__main__":
    pass


# revision 9
# speedup vs baseline: 41.9251x; 41.9251x over previous
"""GATv2 2-layer encoder on 8 TRN2 NeuronCores.

Strategy: destination-node sharding. Nodes are bin-packed into 392 tiles of
128 slots each (balancing in-edge counts), 49 tiles per core. All edges
(incl. self-loops) are grouped by the tile owning their destination; each
tile's edges are padded to BPT blocks of 128. Per edge-block the kernel
gathers xl[src], e[rel], xr[dst] rows (indirect DMA, summed in the DMA
datapath), applies leaky-relu + per-head att dot to get logits, exp (softmax
without max-subtraction — logits are O(1)), and scatter-adds the weighted
source features into the tile's PSUM accumulator with a one-hot matmul.
Segment softmax needs no cross-core traffic; the only collectives are
AllGathers of the per-core node-feature table shards between layers.
"""
import sys
import heapq

import numpy as np

sys.path.insert(0, "/opt/trn_rl_repo")

import ml_dtypes  # noqa: E402
import concourse.bass as bass  # noqa: E402
import concourse.tile as tile  # noqa: E402
from concourse import bacc, mybir  # noqa: E402
from concourse.bass_utils import run_bass_kernel_spmd  # noqa: E402
from concourse.masks import make_identity  # noqa: E402

N, E, R = 50000, 400000, 500
IN, HID, H, OUT = 128, 64, 4, 128
HC1, HC2 = H * HID, H * OUT  # 256, 512
W = 8            # cores
P = 128          # partitions / tile slots / edge-block size
NT = 49          # node tiles per core
TILES = W * NT   # 392
NSLOT = TILES * P  # 50176
SHARD = NT * P   # 6272 rows per core
RPAD = 512       # padded relation table rows (row R = zeros for self-loops)

F32 = mybir.dt.float32
BF16 = mybir.dt.bfloat16
I32 = mybir.dt.int32
BF = ml_dtypes.bfloat16


def _preprocess(edge_index):
    """Self-loops, balanced node->tile binning, per-core block index arrays."""
    src = np.asarray(edge_index[0], dtype=np.int64)
    rel = np.asarray(edge_index[1], dtype=np.int64)
    dst = np.asarray(edge_index[2], dtype=np.int64)
    loop = np.arange(N, dtype=np.int64)
    src_f = np.concatenate([src, loop])
    dst_f = np.concatenate([dst, loop])
    rel_f = np.concatenate([rel, np.full(N, R, dtype=np.int64)])

    deg = np.bincount(dst_f, minlength=N)

    # Greedy balanced binning: highest-degree node to lightest non-full tile.
    order = np.argsort(-deg, kind="stable")
    tile_of = np.empty(N, np.int64)
    slot_of = np.empty(N, np.int64)
    heap = [(0, t) for t in range(TILES)]
    heapq.heapify(heap)
    counts = np.zeros(TILES, np.int64)
    loads = np.zeros(TILES, np.int64)
    for n in order:
        while True:
            load, t = heapq.heappop(heap)
            if counts[t] < P:
                break
        tile_of[n] = t
        slot_of[n] = counts[t]
        counts[t] += 1
        loads[t] += deg[n]
        if counts[t] < P:
            heapq.heappush(heap, (loads[t], t))

    perm_pos = tile_of * P + slot_of  # node -> row in permuted table layout

    bpt = max(1, int(-(-loads.max() // P)))  # blocks per tile (uniform)
    nblk = NT * bpt
    cap = bpt * P

    # Edge slots per tile, padded to cap.
    et = tile_of[dst_f]
    eorder = np.argsort(et, kind="stable")
    et_s = et[eorder]
    starts = np.searchsorted(et_s, np.arange(TILES))
    ends = np.searchsorted(et_s, np.arange(TILES), side="right")

    src_a = np.zeros((TILES, cap), np.int64)
    rel_a = np.full((TILES, cap), R, np.int64)
    dst_a = np.zeros((TILES, cap), np.int64)
    seg_a = np.full((TILES, cap), 999, np.int64)  # 999 => zero Q row (pad)
    for t in range(TILES):
        idx = eorder[starts[t]:ends[t]]
        k = idx.shape[0]
        src_a[t, :k] = src_f[idx]
        rel_a[t, :k] = rel_f[idx]
        dst_a[t, :k] = dst_f[idx]
        seg_a[t, :k] = slot_of[dst_f[idx]]

    # Per-core arrays.
    gidx1 = np.zeros((W, P, nblk * 3), np.int32)
    gidx2 = np.zeros((W, P, nblk * 3), np.int32)
    qh = np.zeros((W, nblk * P, P), BF)
    node_of_slot = np.full(NSLOT, N, np.int64)  # pad slots -> dummy
    for n in range(N):
        node_of_slot[perm_pos[n]] = n
    eye = np.eye(P, dtype=BF)
    zrow = np.zeros(P, BF)
    for c in range(W):
        for t in range(NT):
            g = c * NT + t
            s3 = src_a[g].reshape(bpt, P)
            r3 = rel_a[g].reshape(bpt, P)
            d3 = dst_a[g].reshape(bpt, P)
            sg3 = seg_a[g].reshape(bpt, P)
            for j in range(bpt):
                b = t * bpt + j
                gidx1[c, :, 3 * b + 0] = s3[j]
                gidx1[c, :, 3 * b + 1] = r3[j]
                gidx1[c, :, 3 * b + 2] = d3[j]
                rows = qh[c, b * P:(b + 1) * P]
                valid = sg3[j] < P
                rows[valid] = eye[sg3[j][valid]]
                rows[~valid] = zrow
        # vectorized gidx2 fill
        g0 = c * NT
        s_all = src_a[g0:g0 + NT].reshape(NT * bpt, P)
        r_all = rel_a[g0:g0 + NT].reshape(NT * bpt, P)
        d_all = dst_a[g0:g0 + NT].reshape(NT * bpt, P)
        gidx2[c, :, 0::3] = perm_pos[s_all].T
        gidx2[c, :, 1::3] = r_all.T
        gidx2[c, :, 2::3] = perm_pos[d_all].T

    return dict(
        bpt=bpt, nblk=nblk, perm_pos=perm_pos, node_of_slot=node_of_slot,
        gidx1=gidx1, gidx2=gidx2, qh=qh,
    )


def _build(bpt):
    nblk = NT * bpt
    nc = bacc.Bacc("TRN2", target_bir_lowering=False, debug=False, num_devices=W)

    # ---- per-core inputs
    x_shard = nc.declare_dram_parameter("x_shard", [SHARD, IN], F32, isOutput=False)
    gidx1 = nc.declare_dram_parameter("gidx1", [P, nblk * 3], I32, isOutput=False)
    gidx2 = nc.declare_dram_parameter("gidx2", [P, nblk * 3], I32, isOutput=False)
    qh = nc.declare_dram_parameter("qh", [nblk * P, P], BF16, isOutput=False)
    # ---- replicated inputs
    rel_pad = nc.declare_dram_parameter("rel_pad", [RPAD, IN], F32, isOutput=False)
    wl1 = nc.declare_dram_parameter("wl1", [IN, HC1], F32, isOutput=False)
    wr1 = nc.declare_dram_parameter("wr1", [IN, HC1], F32, isOutput=False)
    we1 = nc.declare_dram_parameter("we1", [IN, HC1], F32, isOutput=False)
    att1f = nc.declare_dram_parameter("att1f", [1, HC1], F32, isOutput=False)
    eb1 = nc.declare_dram_parameter("eb1", [1, HC1], F32, isOutput=False)
    ob1 = nc.declare_dram_parameter("ob1", [1, HC1], F32, isOutput=False)
    wl2 = nc.declare_dram_parameter("wl2", [HC1, HC2], F32, isOutput=False)
    wr2 = nc.declare_dram_parameter("wr2", [HC1, HC2], F32, isOutput=False)
    we2 = nc.declare_dram_parameter("we2", [IN, HC2], F32, isOutput=False)
    att2f = nc.declare_dram_parameter("att2f", [1, HC2], F32, isOutput=False)
    eb2 = nc.declare_dram_parameter("eb2", [1, HC2], F32, isOutput=False)
    ob2 = nc.declare_dram_parameter("ob2", [1, OUT], F32, isOutput=False)
    out_p = nc.declare_dram_parameter("out", [SHARD, OUT], F32, isOutput=True)

    # ---- internal DRAM
    e1t = nc.dram_tensor("e1t", [RPAD, HC1], BF16)
    e2t = nc.dram_tensor("e2t", [RPAD, HC2], BF16)
    xl_shard = nc.dram_tensor("xl_shard", [SHARD, HC1], BF16)
    xr_shard = nc.dram_tensor("xr_shard", [SHARD, HC1], BF16)
    xl1_full = nc.dram_tensor("xl1_full", [NSLOT, HC1], BF16, addr_space="Shared")
    xr1_full = nc.dram_tensor("xr1_full", [NSLOT, HC1], BF16, addr_space="Shared")
    h_shard = nc.dram_tensor("h_shard", [SHARD, HC1], BF16)
    xl2_shard = nc.dram_tensor("xl2_shard", [SHARD, HC2], BF16)
    xr2_shard = nc.dram_tensor("xr2_shard", [SHARD, HC2], BF16)
    xl2_full = nc.dram_tensor("xl2_full", [NSLOT, HC2], BF16, addr_space="Shared")
    xr2_full = nc.dram_tensor("xr2_full", [NSLOT, HC2], BF16, addr_space="Shared")

    RG = [list(range(W))]
    IOA = bass.IndirectOffsetOnAxis

    with tile.TileContext(nc) as tc:
        with (
            tc.tile_pool(name="const", bufs=1) as cp,
            tc.tile_pool(name="work", bufs=3) as wp,
            tc.tile_pool(name="pstp", bufs=2, space="PSUM") as pp_tp,
            tc.tile_pool(name="psb", bufs=1, space="PSUM") as pp_b,
            tc.tile_pool(name="psacc", bufs=2, space="PSUM") as pa,
        ):
            # ================= consts =================
            ident = cp.tile([P, P], BF16)
            make_identity(nc, ident[:])
            wl1b = cp.tile([IN, HC1], BF16, tag="wl1b")
            nc.gpsimd.dma_start(out=wl1b[:], in_=wl1[:])
            wr1b = cp.tile([IN, HC1], BF16, tag="wr1b")
            nc.gpsimd.dma_start(out=wr1b[:], in_=wr1[:])
            we1b = cp.tile([IN, HC1], BF16, tag="we1b")
            nc.gpsimd.dma_start(out=we1b[:], in_=we1[:])
            we2b = cp.tile([IN, HC2], BF16, tag="we2b")
            nc.gpsimd.dma_start(out=we2b[:], in_=we2[:])
            wl2b = []
            wr2b = []
            for k in range(2):
                wl2bk = cp.tile([P, HC2], BF16, tag=f"wl2b{k}")
                nc.gpsimd.dma_start(out=wl2bk[:], in_=wl2[k * P:(k + 1) * P, :])
                wl2b.append(wl2bk)
                wr2bk = cp.tile([P, HC2], BF16, tag=f"wr2b{k}")
                nc.gpsimd.dma_start(out=wr2bk[:], in_=wr2[k * P:(k + 1) * P, :])
                wr2b.append(wr2bk)
            attB1 = cp.tile([P, HC1], BF16, tag="attB1")
            nc.gpsimd.dma_start(out=attB1[:], in_=att1f[:].to_broadcast([P, HC1]))
            attB2 = cp.tile([P, HC2], BF16, tag="attB2")
            nc.gpsimd.dma_start(out=attB2[:], in_=att2f[:].to_broadcast([P, HC2]))
            eb1B = cp.tile([P, HC1], F32, tag="eb1B")
            nc.sync.dma_start(out=eb1B[:], in_=eb1[:].to_broadcast([P, HC1]))
            ob1B = cp.tile([P, HC1], BF16, tag="ob1B")
            nc.gpsimd.dma_start(out=ob1B[:], in_=ob1[:].to_broadcast([P, HC1]))
            eb2B = cp.tile([P, HC2], F32, tag="eb2B")
            nc.sync.dma_start(out=eb2B[:], in_=eb2[:].to_broadcast([P, HC2]))
            ob2B = cp.tile([P, OUT], F32, tag="ob2B")
            nc.sync.dma_start(out=ob2B[:], in_=ob2[:].to_broadcast([P, OUT]))
            gidx1_t = cp.tile([P, nblk * 3], I32, tag="gidx1_t")
            nc.sync.dma_start(out=gidx1_t[:], in_=gidx1[:])
            gidx2_t = cp.tile([P, nblk * 3], I32, tag="gidx2_t")
            nc.sync.dma_start(out=gidx2_t[:], in_=gidx2[:])

            # ================= e-tables =================
            for k in range(RPAD // P):
                rk = wp.tile([P, IN], BF16, tag="rk")
                nc.gpsimd.dma_start(out=rk[:], in_=rel_pad[k * P:(k + 1) * P, :])
                tp = pp_tp.tile([P, P], BF16, tag="tp")
                nc.tensor.transpose(tp[:], rk[:], ident[:])
                rT = wp.tile([P, IN], BF16, tag="rT")
                nc.vector.tensor_copy(rT[:], tp[:])
                psE1 = pp_b.tile([P, HC2], F32, tag="psb")
                nc.tensor.matmul(psE1[:, 0:HC1], lhsT=rT[:], rhs=we1b[:],
                                 start=True, stop=True)
                e1sb = wp.tile([P, HC1], BF16, tag="e1sb")
                nc.vector.tensor_tensor(out=e1sb[:], in0=psE1[:, 0:HC1], in1=eb1B[:],
                                        op=mybir.AluOpType.add)
                nc.sync.dma_start(out=e1t[k * P:(k + 1) * P, :], in_=e1sb[:])
                psE2 = pp_b.tile([P, HC2], F32, tag="psb")
                nc.tensor.matmul(psE2[:], lhsT=rT[:], rhs=we2b[:], start=True, stop=True)
                e2sb = wp.tile([P, HC2], BF16, tag="e2sb")
                nc.vector.tensor_tensor(out=e2sb[:], in0=psE2[:], in1=eb2B[:],
                                        op=mybir.AluOpType.add)
                nc.sync.dma_start(out=e2t[k * P:(k + 1) * P, :], in_=e2sb[:])

            # ================= xl1/xr1 shard build =================
            for t in range(NT):
                xt = wp.tile([P, IN], BF16, tag="xt")
                nc.gpsimd.dma_start(out=xt[:], in_=x_shard[t * P:(t + 1) * P, :])
                tp2 = pp_tp.tile([P, P], BF16, tag="tp")
                nc.tensor.transpose(tp2[:], xt[:], ident[:])
                xT = wp.tile([P, IN], BF16, tag="xT")
                nc.vector.tensor_copy(xT[:], tp2[:])
                psC = pp_b.tile([P, HC2], F32, tag="psb")
                nc.tensor.matmul(psC[:, 0:HC1], lhsT=xT[:], rhs=wl1b[:],
                                 start=True, stop=True)
                nc.tensor.matmul(psC[:, HC1:HC2], lhsT=xT[:], rhs=wr1b[:],
                                 start=True, stop=True)
                xlsb = wp.tile([P, HC1], BF16, tag="xlsb")
                nc.scalar.activation(xlsb[:], psC[:, 0:HC1],
                                     mybir.ActivationFunctionType.Copy)
                nc.sync.dma_start(out=xl_shard[t * P:(t + 1) * P, :], in_=xlsb[:])
                xrsb = wp.tile([P, HC1], BF16, tag="xrsb")
                nc.scalar.activation(xrsb[:], psC[:, HC1:HC2],
                                     mybir.ActivationFunctionType.Copy)
                nc.sync.dma_start(out=xr_shard[t * P:(t + 1) * P, :], in_=xrsb[:])

            nc.gpsimd.collective_compute(
                "AllGather", mybir.AluOpType.bypass,
                ins=[xl_shard[:]], outs=[xl1_full[:]], replica_groups=RG)
            nc.gpsimd.collective_compute(
                "AllGather", mybir.AluOpType.bypass,
                ins=[xr_shard[:]], outs=[xr1_full[:]], replica_groups=RG)

            # ================= layer-1 edges =================
            for t in range(NT):
                acc1 = pa.tile([P, HC1 + 4], F32, tag="accF")
                for j in range(bpt):
                    b = t * bpt + j
                    Qb = wp.tile([P, P], BF16, tag="Qb")
                    nc.sync.dma_start(out=Qb[:], in_=qh[b * P:(b + 1) * P, :])
                    Gl = wp.tile([P, HC1], BF16, tag="Gl")
                    nc.gpsimd.indirect_dma_start(
                        out=Gl[:], out_offset=None, in_=xl1_full[:],
                        in_offset=IOA(ap=gidx1_t[:, 3 * b:3 * b + 1], axis=0))
                    M = wp.tile([P, HC1], BF16, tag="M")
                    nc.gpsimd.indirect_dma_start(
                        out=M[:], out_offset=None, in_=e1t[:],
                        in_offset=IOA(ap=gidx1_t[:, 3 * b + 1:3 * b + 2], axis=0))
                    nc.gpsimd.indirect_dma_start(
                        out=M[:], out_offset=None, in_=xr1_full[:],
                        in_offset=IOA(ap=gidx1_t[:, 3 * b + 2:3 * b + 3], axis=0),
                        compute_op=mybir.AluOpType.add)
                    nc.gpsimd.indirect_dma_start(
                        out=M[:], out_offset=None, in_=xl1_full[:],
                        in_offset=IOA(ap=gidx1_t[:, 3 * b:3 * b + 1], axis=0),
                        compute_op=mybir.AluOpType.add)
                    Mr = wp.tile([P, HC1], BF16, tag="Mr")
                    nc.scalar.activation(Mr[:], M[:],
                                         mybir.ActivationFunctionType.Prelu, alpha=0.2)
                    T = wp.tile([P, HC1], BF16, tag="T")
                    nc.vector.tensor_tensor(out=T[:], in0=Mr[:], in1=attB1[:],
                                            op=mybir.AluOpType.mult)
                    logit = wp.tile([P, H], F32, tag="logit")
                    nc.vector.tensor_reduce(
                        out=logit[:], in_=T[:].rearrange("p (h c) -> p h c", h=H),
                        axis=mybir.AxisListType.X, op=mybir.AluOpType.add)
                    wf = wp.tile([P, H], F32, tag="wf")
                    nc.scalar.activation(wf[:], logit[:],
                                         mybir.ActivationFunctionType.Exp)
                    Rt = wp.tile([P, HC1 + 4], BF16, tag="Rt")
                    nc.scalar.activation(Rt[:, HC1:HC1 + 4], logit[:],
                                         mybir.ActivationFunctionType.Exp)
                    for hh in range(H):
                        nc.scalar.activation(
                            Rt[:, hh * HID:(hh + 1) * HID], Gl[:, hh * HID:(hh + 1) * HID],
                            mybir.ActivationFunctionType.Copy,
                            scale=wf[:, hh:hh + 1])
                    nc.tensor.matmul(acc1[:], lhsT=Qb[:], rhs=Rt[:],
                                     start=(j == 0), stop=(j == bpt - 1))
                # epilogue: h = acc/denom + bias
                rec = wp.tile([P, H], F32, tag="rec")
                nc.vector.reciprocal(rec[:], acc1[:, HC1:HC1 + 4])
                htmp = wp.tile([P, HC1], BF16, tag="htmp")
                for hh in range(H):
                    nc.scalar.activation(
                        htmp[:, hh * HID:(hh + 1) * HID],
                        acc1[:, hh * HID:(hh + 1) * HID],
                        mybir.ActivationFunctionType.Copy, scale=rec[:, hh:hh + 1])
                hsb = wp.tile([P, HC1], BF16, tag="hsb")
                nc.vector.tensor_tensor(out=hsb[:], in0=htmp[:], in1=ob1B[:],
                                        op=mybir.AluOpType.add)
                nc.sync.dma_start(out=h_shard[t * P:(t + 1) * P, :], in_=hsb[:])

            # ================= xl2/xr2 build =================
            for t in range(NT):
                ht = wp.tile([P, HC1], BF16, tag="ht")
                nc.sync.dma_start(out=ht[:], in_=h_shard[t * P:(t + 1) * P, :])
                hT = []
                for k in range(2):
                    tp3 = pp_tp.tile([P, P], BF16, tag="tp")
                    nc.tensor.transpose(tp3[:], ht[:, k * P:(k + 1) * P], ident[:])
                    hTk = wp.tile([P, P], BF16, tag=f"hT{k}")
                    nc.vector.tensor_copy(hTk[:], tp3[:])
                    hT.append(hTk)
                ps2l = pp_b.tile([P, HC2], F32, tag="psb")
                for k in range(2):
                    nc.tensor.matmul(ps2l[:], lhsT=hT[k][:], rhs=wl2b[k][:],
                                     start=(k == 0), stop=(k == 1))
                xl2sb = wp.tile([P, HC2], BF16, tag="xl2sb")
                nc.scalar.activation(xl2sb[:], ps2l[:], mybir.ActivationFunctionType.Copy)
                nc.sync.dma_start(out=xl2_shard[t * P:(t + 1) * P, :], in_=xl2sb[:])
                ps2r = pp_b.tile([P, HC2], F32, tag="psb")
                for k in range(2):
                    nc.tensor.matmul(ps2r[:], lhsT=hT[k][:], rhs=wr2b[k][:],
                                     start=(k == 0), stop=(k == 1))
                xr2sb = wp.tile([P, HC2], BF16, tag="xr2sb")
                nc.scalar.activation(xr2sb[:], ps2r[:], mybir.ActivationFunctionType.Copy)
                nc.sync.dma_start(out=xr2_shard[t * P:(t + 1) * P, :], in_=xr2sb[:])

            nc.gpsimd.collective_compute(
                "AllGather", mybir.AluOpType.bypass,
                ins=[xl2_shard[:]], outs=[xl2_full[:]], replica_groups=RG)
            nc.gpsimd.collective_compute(
                "AllGather", mybir.AluOpType.bypass,
                ins=[xr2_shard[:]], outs=[xr2_full[:]], replica_groups=RG)

            # ================= layer-2 edges =================
            for t in range(NT):
                acc2 = pa.tile([P, HC2], F32, tag="accF")
                accd = pa.tile([P, 4], F32, tag="accD")
                for j in range(bpt):
                    b = t * bpt + j
                    Qb = wp.tile([P, P], BF16, tag="Qb")
                    nc.sync.dma_start(out=Qb[:], in_=qh[b * P:(b + 1) * P, :])
                    Gl2 = wp.tile([P, HC2], BF16, tag="Gl2")
                    nc.gpsimd.indirect_dma_start(
                        out=Gl2[:], out_offset=None, in_=xl2_full[:],
                        in_offset=IOA(ap=gidx2_t[:, 3 * b:3 * b + 1], axis=0))
                    M2 = wp.tile([P, HC2], BF16, tag="M2")
                    nc.gpsimd.indirect_dma_start(
                        out=M2[:], out_offset=None, in_=e2t[:],
                        in_offset=IOA(ap=gidx2_t[:, 3 * b + 1:3 * b + 2], axis=0))
                    nc.gpsimd.indirect_dma_start(
                        out=M2[:], out_offset=None, in_=xr2_full[:],
                        in_offset=IOA(ap=gidx2_t[:, 3 * b + 2:3 * b + 3], axis=0),
                        compute_op=mybir.AluOpType.add)
                    nc.gpsimd.indirect_dma_start(
                        out=M2[:], out_offset=None, in_=xl2_full[:],
                        in_offset=IOA(ap=gidx2_t[:, 3 * b:3 * b + 1], axis=0),
                        compute_op=mybir.AluOpType.add)
                    Mr2 = wp.tile([P, HC2], BF16, tag="Mr2")
                    nc.scalar.activation(Mr2[:], M2[:],
                                         mybir.ActivationFunctionType.Prelu, alpha=0.2)
                    T2 = wp.tile([P, HC2], BF16, tag="T2")
                    nc.vector.tensor_tensor(out=T2[:], in0=Mr2[:], in1=attB2[:],
                                            op=mybir.AluOpType.mult)
                    logit2 = wp.tile([P, H], F32, tag="logit2")
                    nc.vector.tensor_reduce(
                        out=logit2[:], in_=T2[:].rearrange("p (h c) -> p h c", h=H),
                        axis=mybir.AxisListType.X, op=mybir.AluOpType.add)
                    wf2 = wp.tile([P, H], F32, tag="wf2")
                    nc.scalar.activation(wf2[:], logit2[:],
                                         mybir.ActivationFunctionType.Exp)
                    R2 = wp.tile([P, HC2 + 4], BF16, tag="R2")
                    nc.scalar.activation(R2[:, HC2:HC2 + 4], logit2[:],
                                         mybir.ActivationFunctionType.Exp)
                    for hh in range(H):
                        nc.scalar.activation(
                            R2[:, hh * OUT:(hh + 1) * OUT], Gl2[:, hh * OUT:(hh + 1) * OUT],
                            mybir.ActivationFunctionType.Copy,
                            scale=wf2[:, hh:hh + 1])
                    nc.tensor.matmul(acc2[:], lhsT=Qb[:], rhs=R2[:, 0:HC2],
                                     start=(j == 0), stop=(j == bpt - 1))
                    nc.tensor.matmul(accd[:], lhsT=Qb[:], rhs=R2[:, HC2:HC2 + 4],
                                     start=(j == 0), stop=(j == bpt - 1))
                # epilogue: out = mean_h(acc_h/denom_h) + bias
                rec2 = wp.tile([P, H], F32, tag="rec2")
                nc.vector.reciprocal(rec2[:], accd[:])
                rec4 = wp.tile([P, H], F32, tag="rec4")
                nc.vector.tensor_scalar_mul(rec4[:], rec2[:], 0.25)
                hsum = []
                for hh in range(H):
                    ho = wp.tile([P, OUT], F32, tag=f"ho{hh}")
                    nc.scalar.activation(
                        ho[:], acc2[:, hh * OUT:(hh + 1) * OUT],
                        mybir.ActivationFunctionType.Copy, scale=rec4[:, hh:hh + 1])
                    hsum.append(ho)
                s01 = wp.tile([P, OUT], F32, tag="s01")
                nc.vector.tensor_tensor(out=s01[:], in0=hsum[0][:], in1=hsum[1][:],
                                        op=mybir.AluOpType.add)
                s23 = wp.tile([P, OUT], F32, tag="s23")
                nc.vector.tensor_tensor(out=s23[:], in0=hsum[2][:], in1=hsum[3][:],
                                        op=mybir.AluOpType.add)
                s0123 = wp.tile([P, OUT], F32, tag="s0123")
                nc.vector.tensor_tensor(out=s0123[:], in0=s01[:], in1=s23[:],
                                        op=mybir.AluOpType.add)
                osb = wp.tile([P, OUT], F32, tag="osb")
                nc.vector.tensor_tensor(out=osb[:], in0=s0123[:], in1=ob2B[:],
                                        op=mybir.AluOpType.add)
                nc.sync.dma_start(out=out_p[t * P:(t + 1) * P, :], in_=osb[:])

    nc.compile()
    return nc


_CACHE = {}


def kernel(x, edge_index, relations,
           Wl1, bl1, Wr1, br1, We1, att1, bias1,
           Wl2, bl2, Wr2, br2, We2, att2, bias2, **_unused):
    x = np.asarray(x, np.float32)
    edge_index = np.asarray(edge_index)
    relations = np.asarray(relations, np.float32)

    pre = _preprocess(edge_index)
    bpt = pre["bpt"]

    if bpt not in _CACHE:
        _CACHE[bpt] = _build(bpt)
    nc = _CACHE[bpt]

    x_pad = np.zeros((NSLOT, IN), np.float32)
    x_pad[:N] = x
    rel_pad = np.zeros((RPAD, IN), np.float32)
    rel_pad[:R] = relations

    f32 = np.float32
    rep = dict(
        rel_pad=rel_pad,
        wl1=np.asarray(Wl1, f32), wr1=np.asarray(Wr1, f32), we1=np.asarray(We1, f32),
        att1f=np.asarray(att1, f32).reshape(1, HC1),
        eb1=(np.asarray(bl1, f32) + np.asarray(br1, f32)).reshape(1, HC1),
        ob1=(np.asarray(bl1, f32) + np.asarray(bias1, f32)).reshape(1, HC1),
        wl2=np.asarray(Wl2, f32), wr2=np.asarray(Wr2, f32), we2=np.asarray(We2, f32),
        att2f=np.asarray(att2, f32).reshape(1, HC2),
        eb2=(np.asarray(bl2, f32) + np.asarray(br2, f32)).reshape(1, HC2),
        ob2=(np.asarray(bl2, f32).reshape(H, OUT).mean(axis=0)
             + np.asarray(bias2, f32)).reshape(1, OUT),
    )
    in_maps = []
    for c in range(W):
        m = dict(rep)
        m["x_shard"] = np.ascontiguousarray(x_pad[c * SHARD:(c + 1) * SHARD])
        m["gidx1"] = np.ascontiguousarray(pre["gidx1"][c])
        m["gidx2"] = np.ascontiguousarray(pre["gidx2"][c])
        m["qh"] = np.ascontiguousarray(pre["qh"][c])
        in_maps.append(m)

    import os
    trace = os.environ.get("GAT_TRACE", "0") == "1"
    res = run_bass_kernel_spmd(nc, in_maps, list(range(W)), trace=trace)
    global LAST_EXEC_NS, LAST_RES
    LAST_EXEC_NS = res.exec_time_ns
    LAST_RES = res
    cat = np.concatenate([res.results[c]["out"] for c in range(W)], axis=0)
    return np.ascontiguousarray(cat[pre["perm_pos"]])


if __name__ == "__main__":
    pass
